# revision 54
# baseline (speedup 1.0000x reference)
"""AFT-Full (Attention-Free Transformer) distributed Bass kernel for 8 TRN2 NeuronCores.

Reference math (B=4, N=512, D=128):
    q = sigmoid(x @ Wq^T + bq); k = x @ Wk^T + bk; v = x @ Wv^T + bv
    s[b,t,j,d] = k[b,j,d] + pos_bias[t,j];  m = max_j s
    out = q * (sum_j exp(s-m) * v) / (sum_j exp(s-m))

Algebra used:
  * The max-stabilizer m cancels between numerator and denominator, and
    exp(k + pb) = exp(pb) * exp(k): with P = exp(pos_bias), ek = exp(k):
        out = q * (P @ (ek * v)) / (P @ ek)      (matmuls contract over j)
  * The k-bias cancels too: exp(k+bk) = exp(bk)*exp(k) and exp(bk)[d]
    factors out of both j-sums, so k is projected WITHOUT bias.
  * sigmoid(q)/den = 1/(den * (1 + exp(-qlin))), so the epilogue needs only
    Exp activations (one ACT table) and one fast reciprocal:
        out^T = num^T * recip(den^T * (1 + exp(-qlin^T)))
  * bq is applied inside the ACT op (per-partition bias port); bv via a
    rank-1 ones x bv matmul accumulated into the v-half of the PSUM bank.

Sharding: 8 cores = 4 batches x 2 t-halves; no collectives. Each core gets
x[b]^T with its t-half's columns rotated to the front, and pos_bias rows
rotated identically, so the j-contraction order matches and one SPMD graph
serves all cores. Device computes out^T[d, t] for its (b, t-half).

Compute dtype: bf16 into the PE array (fp32 PSUM accumulation); f32->bf16
conversion happens inside SWDGE cast-DMAs on the otherwise-idle GpSimd
engine. A short burst of dummy matmuls warms the PE clock (HAM 1.2->2.4GHz)
while the DMAs are in flight.
"""

import sys

import numpy as np

try:
    import concourse.bass as bass
except ImportError:  # pragma: no cover
    sys.path.insert(0, "/opt/trn_rl_repo")
    import concourse.bass as bass

import concourse.mybir as mybir
import concourse.tile as tile
from concourse import bacc
import concourse.bass_utils as _bu
from concourse.bass_utils import run_bass_kernel_spmd

# Note: walrus pins --enable-ldw-opt=false; enabling it is incompatible with
# Bacc's explicit InstLdweights (move_matmul_waits_to_ldweights), so the
# LDWEIGHTS/MATMUL overlap is not available on this stack.

F32 = mybir.dt.float32
BF16 = mybir.dt.bfloat16
B, N, D = 4, 512, 128
T = N // 2  # t-rows per core
JT = N // 128  # j tiles of 128
AF = mybir.ActivationFunctionType
N_WARMUP_MM = 6


def build_nc() -> bass.Bass:
    # Bacc (not plain Bass): its compile() pass legalizes multi-wait
    # instructions (move_matmul_waits_to_ldweights, event semaphores),
    # which this walrus build requires.
    nc = bacc.Bacc()
    WQ = 3 * D + 1
    XW = WQ + N
    # xw = [Wq^T | Wk^T | Wv^T | bq_col | x^T] packed on columns
    xw = nc.dram_tensor("xw", [D, XW], F32, kind="ExternalInput")
    bv = nc.dram_tensor("bv", [1, D], F32, kind="ExternalInput")
    # pos_bias^T packed so each partition's 4 j-tiles are contiguous
    pbT = nc.dram_tensor("pbT", [128, JT * T], F32, kind="ExternalInput")
    out = nc.dram_tensor("out", [D, T], F32, kind="ExternalOutput")

    with tile.TileContext(nc) as tc:
        with (
            tc.tile_pool(name="sb", bufs=1) as sb,
            tc.tile_pool(name="ps", bufs=1, space="PSUM") as ps,
        ):
            # ---- loads: SP HWDGE ring in consumption order (FIFO per ring):
            # weights+bq, x halves, then pos_bias quarters; bv via tiny
            # SWDGE cast-DMA on GpSimd ----
            # SP HWDGE ring in consumption order: xw (longest dependent
            # chain: cast->kv->ek) first, then pos_bias j0-2, and a small
            # j3 piece last so the post-stream suffix is minimal.
            xwf = sb.tile([D, XW], F32, name="xwf")
            nc.sync.dma_start(xwf[:], xw[:])
            bvb = sb.tile([1, D], BF16, name="bvb")
            nc.gpsimd.dma_start(bvb[:], bv[:])
            pb_sb = sb.tile([128, JT, T], F32, name="pb_sb")
            pb3 = pbT[:].rearrange("p (j t) -> p j t", t=T)
            nc.sync.dma_start(pb_sb[:, 0:3, :], pb3[:, 0:3, :])
            nc.sync.dma_start(pb_sb[:, 3, :], pb3[:, 3, :])

            warm_in = sb.tile([128, N], BF16, name="warm_in")
            nc.vector.memset(warm_in[:], 0.0)
            ones = sb.tile([1, N], BF16, name="ones")
            nc.vector.memset(ones[:], 1.0)
            xwb = sb.tile([D, XW], BF16, name="xwb")
            nc.vector.tensor_copy(xwb[:], xwf[:])
            # -bq as a per-partition column for the ACT bias port
            bqn = sb.tile([D, 1], BF16, name="bqn")
            nc.vector.tensor_scalar_mul(bqn[:], xwb[:, WQ - 1 : WQ], -1.0)
            wb = xwb[:, 0 : 3 * D]
            x0b = xwb[:, WQ : WQ + N // 2]
            x1b = xwb[:, WQ + N // 2 : XW]

            # ---- PE warm-up: dummy matmuls release the HAM clock gate
            # (>=3.4us of sustained PE activity -> 2.4GHz for the real MMs) ----
            warm_ps = ps.tile([128, N], F32, tag="warm_ps")
            for _ in range(N_WARMUP_MM):
                nc.tensor.matmul(
                    warm_ps[:], warm_in[:, 0:128], warm_in[:], start=True, stop=True
                )

            # ---- P^T = exp(pos_bias^T) -> bf16 (j0-2 now, j3 later) ----
            pt = sb.tile([128, JT, T], BF16, name="pt")
            nc.scalar.activation(pt[:, 0:3, :], pb_sb[:, 0:3, :], AF.Exp)

            # ---- k/v projections (k unbiased; bv via rank-1 matmul) ----
            # kv_X[:, j, 0:128] = x_j @ Wk^T ; [..., 128:256] = x_j @ Wv^T + bv
            kv_a = ps.tile([128, 2, 2 * D], F32, tag="kv_a")
            kv_b = ps.tile([128, 2, 2 * D], F32, tag="kv_b")
            kv_ps = [kv_a, kv_b]
            xh = [x0b, x0b, x1b, x1b]

            def kv_pair(j):
                tgt = kv_ps[j // 2][:, j % 2, :]
                # bias into the v-half; start=True clears the whole bank's
                # has_written bits, so the k-half below writes fresh
                nc.tensor.matmul(
                    tgt[:, D : 2 * D], ones[:, 0:128], bvb[:], start=True, stop=False
                )
                nc.tensor.matmul(
                    tgt,
                    xh[j][:, (j % 2) * 128 : (j % 2) * 128 + 128],
                    wb[:, D : 3 * D],
                    start=False, stop=True,
                    skip_group_check=True,
                )

            kv_pair(0)
            kv_pair(1)
            # qlin^T[d,t] = Wq @ x[t-half]^T (bq applied in the ACT below)
            q_ps = ps.tile([D, T], F32, tag="q_ps")
            nc.tensor.matmul(q_ps[:], wb[:, 0:D], x0b[:], start=True, stop=True)
            kv_pair(2)
            kv_pair(3)

            # ---- exp(k), ek*v, and exp(-(qlin+bq)) ----
            ek = sb.tile([128, JT, D], BF16, name="ek")
            wt = sb.tile([128, JT, D], BF16, name="wt")
            nc.scalar.activation(ek[:, 0:2, :], kv_a[:, :, 0:D], AF.Exp)
            nc.vector.tensor_mul(wt[:, 0:2, :], ek[:, 0:2, :], kv_a[:, :, D : 2 * D])
            nc.scalar.activation(ek[:, 2:4, :], kv_b[:, :, 0:D], AF.Exp)
            nc.vector.tensor_mul(wt[:, 2:4, :], ek[:, 2:4, :], kv_b[:, :, D : 2 * D])
            nc.scalar.activation(pt[:, 3, :], pb_sb[:, 3, :], AF.Exp)
            eq = sb.tile([D, T], F32, name="eq")
            nc.scalar.activation(eq[:], q_ps[:], AF.Exp, scale=-1.0, bias=bqn[:])
            g = sb.tile([D, T], F32, name="g")
            nc.vector.tensor_scalar_add(g[:], eq[:], 1.0)

            # ---- den^T = sum_j ek_j @ pt_j ; num^T = sum_j wt_j @ pt_j ----
            # natural j order: the groups close on j3, whose pos_bias piece
            # is the small final DMA
            den_ps = ps.tile([D, T], F32, tag="den_ps")
            num_ps = ps.tile([D, T], F32, tag="num_ps")
            for j in range(JT):
                nc.tensor.matmul(
                    den_ps[:], ek[:, j, :], pt[:, j, :],
                    start=(j == 0), stop=(j == JT - 1),
                )
            for j in range(JT):
                nc.tensor.matmul(
                    num_ps[:], wt[:, j, :], pt[:, j, :],
                    start=(j == 0), stop=(j == JT - 1),
                )

            # ---- out^T = num^T * recip(den^T * g), t-halves pipelined so the
            # first out-DMA's completion latency overlaps the second half ----
            f = sb.tile([D, T], F32, name="f")
            rec = sb.tile([D, T], F32, name="rec")
            out_sb = sb.tile([D, T], F32, name="out_sb")
            half = T // 2
            for h in range(2):
                s = slice(h * half, (h + 1) * half)
                nc.vector.tensor_mul(f[:, s], g[:, s], den_ps[:, s])
                nc.vector.reciprocal_approx_fast(rec[:, s], f[:, s])
                nc.vector.tensor_mul(out_sb[:, s], rec[:, s], num_ps[:, s])
                nc.sync.dma_start(out[:, s], out_sb[:, s])

    _trim_prologue_barrier(nc)
    _trim_epilogue_barrier(nc)
    nc.finalize()
    return nc


def _trim_epilogue_barrier(nc):
    """Keep only the SP tail drain (it carries waits on every semaphore's
    final value, including the output-DMA completion) and drop the two
    all-engine barriers + semaphore range-clear Tile emits after it. The
    runtime's own per-engine teardown then starts right after each engine's
    last kernel instruction instead of waiting for the slowest engine, and
    the runtime's start-sequence barrier keeps re-execution ordered."""
    for f in nc.m.functions:
        for blk in f.blocks:
            if not blk.name.endswith("_end"):
                continue
            keep = []
            past_drain = False
            for inst in blk.instructions:
                tn = type(inst).__name__
                if not past_drain:
                    keep.append(inst)
                    if tn == "InstDrain":
                        past_drain = True  # the SP tail drain; drop the rest
            blk.instructions[:] = keep


def _trim_prologue_barrier(nc):
    """Drop Bass.__init__'s const-AP barrier and the dead const memsets from
    the main block. The only live const (float32-0.0, read by ACT bias many
    microseconds later) is written by GpSimd before its first DMA emission,
    so the all-engine barrier only delays the first loads by ~1us."""
    blk = nc.m.functions[0].blocks[0]
    keep = []
    for inst in blk.instructions:
        tn = type(inst).__name__
        if tn in ("InstDrain", "InstEventSemaphore"):
            continue
        if tn == "InstMemset":
            tgt = str(inst.outs[0].memref) if inst.outs else ""
            if "const-" in tgt and "float32-0" not in tgt:
                continue
        keep.append(inst)
    blk.instructions[:] = keep


def prepare_in_maps(x, Wq, bq, Wk, bk, Wv, bv, pos_bias):
    x = np.asarray(x, dtype=np.float32)
    pos_bias = np.asarray(pos_bias, dtype=np.float32)
    wcols = np.concatenate(
        [
            np.asarray(Wq, np.float32).T,
            np.asarray(Wk, np.float32).T,
            np.asarray(Wv, np.float32).T,
            np.asarray(bq, np.float32)[:, None],
        ],
        axis=1,
    )
    bv_row = np.ascontiguousarray(np.asarray(bv, np.float32)[None])
    bk = np.asarray(bk, np.float32)  # unused on device: exp(bk) cancels

    in_maps = []
    for i in range(8):
        b, th = divmod(i, 2)
        t0 = th * T
        perm = np.concatenate([np.arange(t0, N), np.arange(0, t0)])
        xT = x[b][perm].T  # [128, 512]
        pb = pos_bias[t0 : t0 + T][:, perm].T  # [512, 256] (j, t)
        # pack so each SBUF partition's 4 j-tiles are contiguous: [128, 4*256]
        pb2 = np.ascontiguousarray(
            pb.reshape(JT, 128, T).transpose(1, 0, 2).reshape(128, JT * T)
        )
        in_maps.append(
            {
                "xw": np.ascontiguousarray(np.concatenate([wcols, xT], axis=1)),
                "bv": bv_row,
                "pbT": pb2,
            }
        )
    return in_maps


def assemble_output(results) -> np.ndarray:
    out = np.empty((B, N, D), np.float32)
    for i in range(8):
        b, th = divmod(i, 2)
        t0 = th * T
        out[b, t0 : t0 + T, :] = results[i]["out"].T
    return out


def kernel(x, Wq, bq, Wk, bk, Wv, bv, pos_bias) -> np.ndarray:
    in_maps = prepare_in_maps(x, Wq, bq, Wk, bk, Wv, bv, pos_bias)
    nc = build_nc()
    res = run_bass_kernel_spmd(nc, in_maps, core_ids=list(range(8))).results
    return assemble_output(res)


if __name__ == "__main__":
    rng = np.random.default_rng(0)
    s = 1.0 / np.sqrt(D)
    inputs = dict(
        x=rng.standard_normal((B, N, D), dtype=np.float32),
        Wq=rng.standard_normal((D, D), dtype=np.float32) * s,
        bq=rng.standard_normal((D,), dtype=np.float32) * s,
        Wk=rng.standard_normal((D, D), dtype=np.float32) * s,
        bk=rng.standard_normal((D,), dtype=np.float32) * s,
        Wv=rng.standard_normal((D, D), dtype=np.float32) * s,
        bv=rng.standard_normal((D,), dtype=np.float32) * s,
        pos_bias=rng.standard_normal((N, N), dtype=np.float32) * 0.1,
    )
    out = kernel(**inputs)
    print("kernel ran, out shape:", out.shape)


# revision 55
# speedup vs baseline: 1.0831x; 1.0831x over previous
"""AFT-Full (Attention-Free Transformer) distributed Bass kernel for 8 TRN2 NeuronCores.

Reference math (B=4, N=512, D=128):
    q = sigmoid(x @ Wq^T + bq); k = x @ Wk^T + bk; v = x @ Wv^T + bv
    s[b,t,j,d] = k[b,j,d] + pos_bias[t,j];  m = max_j s
    out = q * (sum_j exp(s-m) * v) / (sum_j exp(s-m))

Algebra used:
  * The max-stabilizer m cancels between numerator and denominator, and
    exp(k + pb) = exp(pb) * exp(k): with P = exp(pos_bias), ek = exp(k):
        out = q * (P @ (ek * v)) / (P @ ek)      (matmuls contract over j)
  * The k-bias cancels too: exp(k+bk) = exp(bk)*exp(k) and exp(bk)[d]
    factors out of both j-sums, so k is projected WITHOUT bias.
  * sigmoid(q)/den = 1/(den * (1 + exp(-qlin))), so the epilogue needs only
    Exp activations (one ACT table) and one fast reciprocal:
        out^T = num^T * recip(den^T * (1 + exp(-qlin^T)))
  * bq is applied inside the ACT op (per-partition bias port); bv via a
    rank-1 ones x bv matmul accumulated into the v-half of the PSUM bank.

Sharding: 8 cores = 4 batches x 2 t-halves; no collectives. Each core gets
x[b]^T with its t-half's columns rotated to the front, and pos_bias rows
rotated identically, so the j-contraction order matches and one SPMD graph
serves all cores. Device computes out^T[d, t] for its (b, t-half).

Compute dtype: bf16 into the PE array (fp32 PSUM accumulation); f32->bf16
conversion happens inside SWDGE cast-DMAs on the otherwise-idle GpSimd
engine. A short burst of dummy matmuls warms the PE clock (HAM 1.2->2.4GHz)
while the DMAs are in flight.
"""

import sys

import numpy as np

try:
    import concourse.bass as bass
except ImportError:  # pragma: no cover
    sys.path.insert(0, "/opt/trn_rl_repo")
    import concourse.bass as bass

import concourse.mybir as mybir
import concourse.tile as tile
from concourse import bacc
import concourse.bass_utils as _bu
from concourse.bass_utils import run_bass_kernel_spmd

# Note: walrus pins --enable-ldw-opt=false; enabling it is incompatible with
# Bacc's explicit InstLdweights (move_matmul_waits_to_ldweights), so the
# LDWEIGHTS/MATMUL overlap is not available on this stack.

F32 = mybir.dt.float32
BF16 = mybir.dt.bfloat16
B, N, D = 4, 512, 128
T = N // 2  # t-rows per core
JT = N // 128  # j tiles of 128
AF = mybir.ActivationFunctionType
N_WARMUP_MM = 6


def build_nc() -> bass.Bass:
    # Bacc (not plain Bass): its compile() pass legalizes multi-wait
    # instructions (move_matmul_waits_to_ldweights, event semaphores),
    # which this walrus build requires.
    nc = bacc.Bacc()
    WQ = 3 * D + 1
    XW = WQ + N
    # xw = [Wq^T | Wk^T | Wv^T | bq_col | x^T] packed on columns
    xw = nc.dram_tensor("xw", [D, XW], F32, kind="ExternalInput")
    bv = nc.dram_tensor("bv", [1, D], F32, kind="ExternalInput")
    # pos_bias^T packed so each partition's 4 j-tiles are contiguous
    pbT = nc.dram_tensor("pbT", [128, JT * T], F32, kind="ExternalInput")
    out = nc.dram_tensor("out", [D, T], F32, kind="ExternalOutput")

    with tile.TileContext(nc) as tc:
        with (
            tc.tile_pool(name="sb", bufs=1) as sb,
            tc.tile_pool(name="ps", bufs=1, space="PSUM") as ps,
        ):
            # ---- loads: SP HWDGE ring in consumption order (FIFO per ring):
            # weights+bq, x halves, then pos_bias quarters; bv via tiny
            # SWDGE cast-DMA on GpSimd ----
            # SP HWDGE ring in consumption order: xw (longest dependent
            # chain: cast->kv->ek) first, then pos_bias j0-2, and a small
            # j3 piece last so the post-stream suffix is minimal.
            xwf = sb.tile([D, XW], F32, name="xwf")
            nc.sync.dma_start(xwf[:], xw[:])
            bvb = sb.tile([1, D], BF16, name="bvb")
            nc.gpsimd.dma_start(bvb[:], bv[:])
            pb_sb = sb.tile([128, JT, T], F32, name="pb_sb")
            pb3 = pbT[:].rearrange("p (j t) -> p j t", t=T)
            nc.sync.dma_start(pb_sb[:, 0:3, :], pb3[:, 0:3, :])
            nc.sync.dma_start(pb_sb[:, 3, :], pb3[:, 3, :])

            warm_in = sb.tile([128, N], BF16, name="warm_in")
            nc.vector.memset(warm_in[:], 0.0)
            ones = sb.tile([1, N], BF16, name="ones")
            nc.vector.memset(ones[:], 1.0)
            xwb = sb.tile([D, XW], BF16, name="xwb")
            nc.vector.tensor_copy(xwb[:], xwf[:])
            # -bq as a per-partition column for the ACT bias port
            bqn = sb.tile([D, 1], BF16, name="bqn")
            nc.vector.tensor_scalar_mul(bqn[:], xwb[:, WQ - 1 : WQ], -1.0)
            wb = xwb[:, 0 : 3 * D]
            x0b = xwb[:, WQ : WQ + N // 2]
            x1b = xwb[:, WQ + N // 2 : XW]

            # ---- PE warm-up: dummy matmuls release the HAM clock gate
            # (>=3.4us of sustained PE activity -> 2.4GHz for the real MMs) ----
            warm_ps = ps.tile([128, N], F32, tag="warm_ps")
            for _ in range(N_WARMUP_MM):
                nc.tensor.matmul(
                    warm_ps[:], warm_in[:, 0:128], warm_in[:], start=True, stop=True
                )

            # ---- P^T = exp(pos_bias^T) -> bf16 (j0-2 now, j3 later) ----
            pt = sb.tile([128, JT, T], BF16, name="pt")
            nc.scalar.activation(pt[:, 0:3, :], pb_sb[:, 0:3, :], AF.Exp)

            # ---- k/v projections (k unbiased; bv via rank-1 matmul) ----
            # kv_X[:, j, 0:128] = x_j @ Wk^T ; [..., 128:256] = x_j @ Wv^T + bv
            kv_a = ps.tile([128, 2, 2 * D], F32, tag="kv_a")
            kv_b = ps.tile([128, 2, 2 * D], F32, tag="kv_b")
            kv_ps = [kv_a, kv_b]
            xh = [x0b, x0b, x1b, x1b]

            def kv_pair(j):
                tgt = kv_ps[j // 2][:, j % 2, :]
                # bias into the v-half; start=True clears the whole bank's
                # has_written bits, so the k-half below writes fresh
                nc.tensor.matmul(
                    tgt[:, D : 2 * D], ones[:, 0:128], bvb[:], start=True, stop=False
                )
                nc.tensor.matmul(
                    tgt,
                    xh[j][:, (j % 2) * 128 : (j % 2) * 128 + 128],
                    wb[:, D : 3 * D],
                    start=False, stop=True,
                    skip_group_check=True,
                )

            kv_pair(0)
            kv_pair(1)
            # qlin^T[d,t] = Wq @ x[t-half]^T (bq applied in the ACT below)
            q_ps = ps.tile([D, T], F32, tag="q_ps")
            nc.tensor.matmul(q_ps[:], wb[:, 0:D], x0b[:], start=True, stop=True)
            kv_pair(2)
            kv_pair(3)

            # ---- exp(k), ek*v, and exp(-(qlin+bq)) ----
            ek = sb.tile([128, JT, D], BF16, name="ek")
            wt = sb.tile([128, JT, D], BF16, name="wt")
            nc.scalar.activation(ek[:, 0:2, :], kv_a[:, :, 0:D], AF.Exp)
            nc.vector.tensor_mul(wt[:, 0:2, :], ek[:, 0:2, :], kv_a[:, :, D : 2 * D])
            nc.scalar.activation(ek[:, 2:4, :], kv_b[:, :, 0:D], AF.Exp)
            nc.vector.tensor_mul(wt[:, 2:4, :], ek[:, 2:4, :], kv_b[:, :, D : 2 * D])
            nc.scalar.activation(pt[:, 3, :], pb_sb[:, 3, :], AF.Exp)
            eq = sb.tile([D, T], F32, name="eq")
            nc.scalar.activation(eq[:], q_ps[:], AF.Exp, scale=-1.0, bias=bqn[:])
            g = sb.tile([D, T], F32, name="g")
            nc.vector.tensor_scalar_add(g[:], eq[:], 1.0)

            # ---- den^T = sum_j ek_j @ pt_j ; num^T = sum_j wt_j @ pt_j ----
            # natural j order: the groups close on j3, whose pos_bias piece
            # is the small final DMA
            den_ps = ps.tile([D, T], F32, tag="den_ps")
            num_ps = ps.tile([D, T], F32, tag="num_ps")
            for j in range(JT):
                nc.tensor.matmul(
                    den_ps[:], ek[:, j, :], pt[:, j, :],
                    start=(j == 0), stop=(j == JT - 1),
                )
            for j in range(JT):
                nc.tensor.matmul(
                    num_ps[:], wt[:, j, :], pt[:, j, :],
                    start=(j == 0), stop=(j == JT - 1),
                )

            # ---- out^T = num^T * recip(den^T * g), t-halves pipelined so the
            # first out-DMA's completion latency overlaps the second half ----
            f = sb.tile([D, T], F32, name="f")
            rec = sb.tile([D, T], F32, name="rec")
            out_sb = sb.tile([D, T], F32, name="out_sb")
            half = T // 2
            for h in range(2):
                s = slice(h * half, (h + 1) * half)
                nc.vector.tensor_mul(f[:, s], g[:, s], den_ps[:, s])
                nc.vector.reciprocal_approx_fast(rec[:, s], f[:, s])
                nc.vector.tensor_mul(out_sb[:, s], rec[:, s], num_ps[:, s])
                nc.sync.dma_start(out[:, s], out_sb[:, s])

    _trim_prologue_barrier(nc)
    _trim_epilogue_barrier(nc)
    nc.finalize()
    return nc


def _trim_epilogue_barrier(nc):
    """Keep only the SP tail drain (it carries waits on every semaphore's
    final value, including the output-DMA completion) and drop the two
    all-engine barriers + semaphore range-clear Tile emits after it. The
    runtime's own per-engine teardown then starts right after each engine's
    last kernel instruction instead of waiting for the slowest engine, and
    the runtime's start-sequence barrier keeps re-execution ordered."""
    for f in nc.m.functions:
        for blk in f.blocks:
            if not blk.name.endswith("_end"):
                continue
            keep = []
            past_clear = False
            for inst in blk.instructions:
                if not past_clear:
                    keep.append(inst)
                    if type(inst).__name__ == "InstISA":
                        past_clear = True  # sem range-clear; drop barrier #2
            blk.instructions[:] = keep


def _trim_prologue_barrier(nc):
    """Drop Bass.__init__'s const-AP barrier and the dead const memsets from
    the main block. The only live const (float32-0.0, read by ACT bias many
    microseconds later) is written by GpSimd before its first DMA emission,
    so the all-engine barrier only delays the first loads by ~1us."""
    blk = nc.m.functions[0].blocks[0]
    keep = []
    for inst in blk.instructions:
        tn = type(inst).__name__
        if tn in ("InstDrain", "InstEventSemaphore"):
            continue
        if tn == "InstMemset":
            tgt = str(inst.outs[0].memref) if inst.outs else ""
            if "const-" in tgt and "float32-0" not in tgt:
                continue
        keep.append(inst)
    blk.instructions[:] = keep


def prepare_in_maps(x, Wq, bq, Wk, bk, Wv, bv, pos_bias):
    x = np.asarray(x, dtype=np.float32)
    pos_bias = np.asarray(pos_bias, dtype=np.float32)
    wcols = np.concatenate(
        [
            np.asarray(Wq, np.float32).T,
            np.asarray(Wk, np.float32).T,
            np.asarray(Wv, np.float32).T,
            np.asarray(bq, np.float32)[:, None],
        ],
        axis=1,
    )
    bv_row = np.ascontiguousarray(np.asarray(bv, np.float32)[None])
    bk = np.asarray(bk, np.float32)  # unused on device: exp(bk) cancels

    in_maps = []
    for i in range(8):
        b, th = divmod(i, 2)
        t0 = th * T
        perm = np.concatenate([np.arange(t0, N), np.arange(0, t0)])
        xT = x[b][perm].T  # [128, 512]
        pb = pos_bias[t0 : t0 + T][:, perm].T  # [512, 256] (j, t)
        # pack so each SBUF partition's 4 j-tiles are contiguous: [128, 4*256]
        pb2 = np.ascontiguousarray(
            pb.reshape(JT, 128, T).transpose(1, 0, 2).reshape(128, JT * T)
        )
        in_maps.append(
            {
                "xw": np.ascontiguousarray(np.concatenate([wcols, xT], axis=1)),
                "bv": bv_row,
                "pbT": pb2,
            }
        )
    return in_maps


def assemble_output(results) -> np.ndarray:
    out = np.empty((B, N, D), np.float32)
    for i in range(8):
        b, th = divmod(i, 2)
        t0 = th * T
        out[b, t0 : t0 + T, :] = results[i]["out"].T
    return out


def kernel(x, Wq, bq, Wk, bk, Wv, bv, pos_bias) -> np.ndarray:
    in_maps = prepare_in_maps(x, Wq, bq, Wk, bk, Wv, bv, pos_bias)
    nc = build_nc()
    res = run_bass_kernel_spmd(nc, in_maps, core_ids=list(range(8))).results
    return assemble_output(res)


if __name__ == "__main__":
    rng = np.random.default_rng(0)
    s = 1.0 / np.sqrt(D)
    inputs = dict(
        x=rng.standard_normal((B, N, D), dtype=np.float32),
        Wq=rng.standard_normal((D, D), dtype=np.float32) * s,
        bq=rng.standard_normal((D,), dtype=np.float32) * s,
        Wk=rng.standard_normal((D, D), dtype=np.float32) * s,
        bk=rng.standard_normal((D,), dtype=np.float32) * s,
        Wv=rng.standard_normal((D, D), dtype=np.float32) * s,
        bv=rng.standard_normal((D,), dtype=np.float32) * s,
        pos_bias=rng.standard_normal((N, N), dtype=np.float32) * 0.1,
    )
    out = kernel(**inputs)
    print("kernel ran, out shape:", out.shape)
